# revision 6
# baseline (speedup 1.0000x reference)
"""Bass/Trainium2 SPMD kernel for nn_Network_49589692400277.

LSTM (B=16, L=1024, D=2048, H=512) + argmax-gated exclusive prefix/suffix
sums. Batch-sharded across 8 NeuronCores (2 batches per core).

Per-core device program, 3 phases:
  1. xg = x @ W_ih.T + b          (PE, token-tiled; XG resident in SBUF bf16)
  2. 1024-step recurrence         (~25 instructions per step)
  3. classifier gate + gated prefix/suffix scans via triangular matmuls.

Scaling convention on device: state stored as h-hat = h/2 (bf16); W_hh,
W_lin-diff, att pre-scaled x2 on host; g-gate rows additionally x2 so
tanh(g) = 2*sigmoid(2g) - 1 needs only sigmoids.
"""

import sys

sys.path.insert(0, "/opt/trn_rl_repo")

import numpy as np
import ml_dtypes

import concourse.bass as bass
import concourse.tile as tile
from concourse import bacc, mybir
from concourse import bass_utils

dt = mybir.dt
bf16 = ml_dtypes.bfloat16
NCORES = 8
B, L, D, H = 16, 1024, 2048, 512
Q = 4 * H
BL = 2
TOK = BL * L

_NC_CACHE = {}


def _alu(name):
    return getattr(mybir.AluOpType, name)


def build_nc(bdiff: float, seq_len: int = L):
    nc = bacc.Bacc("TRN2", target_bir_lowering=False, debug=False, num_devices=NCORES)
    Lc = seq_len
    TOKc = BL * Lc
    n_ttiles = TOKc // 128
    n_mtiles = Lc // 128

    xT = nc.dram_tensor("xT", [D, TOKc], dt.bfloat16, kind="ExternalInput").ap()
    wih = nc.dram_tensor("wih", [D, Q], dt.bfloat16, kind="ExternalInput").ap()
    whh = nc.dram_tensor("whh", [H, Q], dt.bfloat16, kind="ExternalInput").ap()
    biasq = nc.dram_tensor("biasq", [1, Q], dt.bfloat16, kind="ExternalInput").ap()
    ones1 = nc.dram_tensor("ones1", [1, 128], dt.bfloat16, kind="ExternalInput").ap()
    wdrep = nc.dram_tensor("wdrep", [128, H], dt.bfloat16, kind="ExternalInput").ap()
    attrep = nc.dram_tensor("attrep", [128, 2 * H], dt.float32, kind="ExternalInput").ap()
    tri = nc.dram_tensor("tri", [128, 384], dt.bfloat16, kind="ExternalInput").ap()
    eye = nc.dram_tensor("eye", [128, 128], dt.bfloat16, kind="ExternalInput").ap()
    out = nc.dram_tensor("out", [BL, Lc, H], dt.float32, kind="ExternalOutput").ap()

    hb0 = nc.dram_tensor("hb0", [BL * H], dt.bfloat16, kind="Internal")
    hb1 = nc.dram_tensor("hb1", [BL * H], dt.bfloat16, kind="Internal")
    gb = nc.dram_tensor("gb", [2 * Lc], dt.float32, kind="Internal")

    # raw sbuf tensor for the transposed state history (custom APs needed)
    lstm_h = nc.alloc_sbuf_tensor("lstm_hist", [128, 8 * Lc], dt.bfloat16)
    lstm = lstm_h.ap()
    lstm3 = lstm.rearrange("p (t x) -> p t x", x=8)

    with tile.TileContext(nc) as tc:
        with (
            tc.tile_pool(name="persist", bufs=1) as pers,
            tc.tile_pool(name="psum", bufs=1, space="PSUM") as pp,
        ):
            # ---------- persistent tensors ----------
            xg_tiles = []
            for i in range(n_ttiles):
                xg_i = pers.tile([128, Q], dt.bfloat16, tag=f"xg{i}")
                xg_tiles.append(xg_i)
            whh_sb = []
            for k in range(4):
                wk = pers.tile([128, Q], dt.bfloat16, tag=f"whh{k}")
                nc.sync.dma_start(wk[:], whh[128 * k : 128 * (k + 1), :])
                whh_sb.append(wk)
            ones_sb = pers.tile([1, 128], dt.bfloat16, tag="ones1")
            nc.sync.dma_start(ones_sb[:], ones1[:])
            bias_sb = pers.tile([1, Q], dt.bfloat16, tag="biasq")
            nc.sync.dma_start(bias_sb[:], biasq[:])
            wdrep_sb = pers.tile([128, H], dt.bfloat16, tag="wdrep")
            nc.sync.dma_start(wdrep_sb[:], wdrep[:])
            attrep_sb = pers.tile([128, 2 * H], dt.float32, tag="attrep")
            nc.sync.dma_start(attrep_sb[:], attrep[:])
            tri_sb = pers.tile([128, 384], dt.bfloat16, tag="tri")
            nc.sync.dma_start(tri_sb[:], tri[:])
            eye_sb = pers.tile([128, 128], dt.bfloat16, tag="eye")
            nc.sync.dma_start(eye_sb[:], eye[:])
            c_sb = pers.tile([BL, H], dt.float32, tag="c")
            nc.vector.memset(c_sb[:], 0.0)

            # ---------- phase 1 ----------
            with (
                tc.tile_pool(name="ph1w", bufs=1) as ph1w,
                tc.tile_pool(name="ph1x", bufs=2) as ph1x,
            ):
                wih_sb = []
                for k in range(D // 128):
                    wt = ph1w.tile([128, Q], dt.bfloat16, tag=f"wih{k}")
                    nc.sync.dma_start(wt[:], wih[128 * k : 128 * (k + 1), :])
                    wih_sb.append(wt)
                CH = 256
                for ch in range(TOKc // CH):
                    xch = ph1x.tile([128, CH * (D // 128)], dt.bfloat16, tag="xch")
                    for k in range(D // 128):
                        nc.sync.dma_start(
                            xch[:, CH * k : CH * (k + 1)],
                            xT[128 * k : 128 * (k + 1), CH * ch : CH * (ch + 1)],
                        )
                    for mt in range(CH // 128):
                        gt = 2 * ch + mt
                        ps = pp.tile([128, Q], dt.float32, tag="AB"[mt])
                        for n in range(Q // 512):
                            for k in range(D // 128):
                                nc.tensor.matmul(
                                    ps[:, 512 * n : 512 * (n + 1)],
                                    xch[:, CH * k + 128 * mt : CH * k + 128 * (mt + 1)],
                                    wih_sb[k][:, 512 * n : 512 * (n + 1)],
                                    start=(k == 0),
                                    stop=False,
                                )
                            nc.tensor.matmul(
                                ps[:, 512 * n : 512 * (n + 1)],
                                ones_sb[:],
                                bias_sb[:, 512 * n : 512 * (n + 1)],
                                start=False,
                                stop=True,
                            )
                        nc.scalar.copy(xg_tiles[gt][:], ps[:])

            # ---------- phases 2+3 ----------
            with (
                tc.tile_pool(name="xgt", bufs=2) as xgtp,
                tc.tile_pool(name="steps", bufs=1) as stp,
                tc.tile_pool(name="small", bufs=2) as smallp,
                tc.tile_pool(name="ph3", bufs=1) as ph3,
            ):
                hbs = [hb0, hb1]
                for t in range(Lc):
                    ps = pp.tile([BL, Q], dt.float32, tag="AB"[t % 2])
                    xg_t = xgtp.tile([BL, Q], dt.bfloat16, tag="xg_t")
                    row = (2 * t) % 128
                    nc.sync.dma_start(xg_t[:], xg_tiles[t // 64][row : row + 2, :])
                    if t > 0:
                        base = 8 * (t - 1)
                        for n in range(Q // 512):
                            for k in range(4):
                                nc.tensor.matmul(
                                    ps[:, 512 * n : 512 * (n + 1)],
                                    lstm[:, base + 2 * k : base + 2 * k + 2],
                                    whh_sb[k][:, 512 * n : 512 * (n + 1)],
                                    start=(k == 0),
                                    stop=(k == 3),
                                )
                        gsum = stp.tile([BL, Q], dt.float32, tag="gsum")
                        nc.vector.tensor_tensor(gsum[:], ps[:], xg_t[:], _alu("add"))
                        sg_in = gsum
                    else:
                        sg_in = xg_t
                    sg = stp.tile([BL, Q], dt.float32, tag="sg")
                    nc.scalar.activation(
                        sg[:], sg_in[:], mybir.ActivationFunctionType.Sigmoid
                    )
                    ig2 = stp.tile([BL, H], dt.float32, tag="ig2")
                    nc.vector.scalar_tensor_tensor(
                        ig2[:], sg[:, 2 * H : 3 * H], 0.5, sg[:, 0:H],
                        _alu("subtract"), _alu("mult"),
                    )
                    fc = stp.tile([BL, H], dt.float32, tag="fc")
                    nc.vector.tensor_tensor(
                        fc[:], sg[:, H : 2 * H], c_sb[:], _alu("mult")
                    )
                    nc.vector.scalar_tensor_tensor(
                        c_sb[:], ig2[:], 2.0, fc[:], _alu("mult"), _alu("add")
                    )
                    s2c = stp.tile([BL, H], dt.float32, tag="s2c")
                    nc.scalar.activation(
                        s2c[:], c_sb[:], mybir.ActivationFunctionType.Sigmoid,
                        scale=2.0,
                    )
                    hh = stp.tile([BL, H], dt.bfloat16, tag="hh")
                    hh_perm = hh[:].rearrange("b (p s) -> b s p", p=128, s=4)
                    nc.vector.scalar_tensor_tensor(
                        hh_perm, s2c[:], 0.5, sg[:, 3 * H : 4 * H],
                        _alu("subtract"), _alu("mult"),
                    )
                    hbd = hbs[t % 2]
                    nc.sync.dma_start(bass.AP(hbd, 0, [[1, BL], [2, H]]), hh[:])
                    nc.sync.dma_start(
                        lstm[:, 8 * t : 8 * t + 8],
                        bass.AP(hbd, 0, [[8, 128], [1, 8]]),
                    )

                # ---------- phase 3 ----------
                dps = pp.tile([128, 2 * Lc], dt.float32, tag="A")
                ch3 = min(512, 2 * Lc)
                tch3 = ch3 // 2
                for n in range((2 * Lc) // ch3):
                    for su in range(4):
                        src = lstm3[:, tch3 * n : tch3 * (n + 1), 2 * su : 2 * su + 2]
                        nc.tensor.matmul(
                            dps[:, ch3 * n : ch3 * (n + 1)],
                            wdrep_sb[:, 128 * su : 128 * (su + 1)],
                            src,
                            start=(su == 0),
                            stop=(su == 3),
                        )
                g_sb = smallp.tile([128, 2 * Lc], dt.float32, tag="g")
                nc.vector.tensor_scalar(
                    g_sb[:], dps[:], float(-bdiff), None, _alu("is_gt")
                )
                nc.sync.dma_start(
                    bass.AP(gb, 0, [[1, 1], [1, 2 * Lc]]), g_sb[0:1, :]
                )
                gT = smallp.tile([128, 2 * n_mtiles], dt.float32, tag="gT")
                nc.sync.dma_start(
                    gT[:], bass.AP(gb, 0, [[2, 128], [256, n_mtiles], [1, 2]])
                )
                lstmB, glB = {}, {}
                for b in range(BL):
                    for tt in range(n_mtiles):
                        pst = pp.tile([128, H], dt.bfloat16, tag="B")
                        for su in range(4):
                            src = lstm3[:, 128 * tt : 128 * (tt + 1), 2 * su + b]
                            nc.tensor.transpose(
                                pst[:, 128 * su : 128 * (su + 1)], src, eye_sb[:]
                            )
                        lb = ph3.tile([128, H], dt.bfloat16, tag=f"lB{b}_{tt}")
                        nc.scalar.copy(lb[:], pst[:])
                        lstmB[(b, tt)] = lb
                        gl = ph3.tile([128, H], dt.bfloat16, tag=f"gB{b}_{tt}")
                        nc.scalar.activation(
                            gl[:], lb[:], mybir.ActivationFunctionType.Copy,
                            scale=gT[:, 2 * tt + b : 2 * tt + b + 1],
                        )
                        glB[(b, tt)] = gl
                TRIU = tri_sb[:, 0:128]
                TRIL = tri_sb[:, 128:256]
                ONES = tri_sb[:, 256:384]
                for b in range(BL):
                    for m in range(n_mtiles):
                        psF = pp.tile([128, H], dt.float32, tag="A")
                        psB = pp.tile([128, H], dt.float32, tag="B")
                        for k in range(0, m + 1):
                            nc.tensor.matmul(
                                psF[:], ONES if k < m else TRIU, glB[(b, k)][:],
                                start=(k == 0), stop=(k == m),
                            )
                        for k in range(m, n_mtiles):
                            nc.tensor.matmul(
                                psB[:], ONES if k > m else TRIL, glB[(b, k)][:],
                                start=(k == m), stop=(k == n_mtiles - 1),
                            )
                        o1 = smallp.tile([128, H], dt.float32, tag="o1")
                        nc.vector.tensor_tensor(
                            o1[:], psF[:], attrep_sb[:, 0:H], _alu("mult")
                        )
                        o2 = smallp.tile([128, H], dt.float32, tag="o2")
                        nc.vector.tensor_tensor(
                            o2[:], psB[:], attrep_sb[:, H : 2 * H], _alu("mult")
                        )
                        o3 = smallp.tile([128, H], dt.float32, tag="o3")
                        nc.vector.scalar_tensor_tensor(
                            o3[:], lstmB[(b, m)][:], 4.0, o1[:],
                            _alu("mult"), _alu("add"),
                        )
                        res = smallp.tile([128, H], dt.float32, tag="res")
                        nc.vector.tensor_tensor(res[:], o3[:], o2[:], _alu("add"))
                        nc.sync.dma_start(
                            out[b, 128 * m : 128 * (m + 1), :], res[:]
                        )

    nc.compile()
    return nc


def _prep_inputs(x, W_ih, W_hh, b_ih, b_hh, W_lin, b_lin, fwd_att, bwd_att):
    f32 = np.float32
    x = np.asarray(x, f32)
    W_ih = np.asarray(W_ih, f32)
    W_hh = np.asarray(W_hh, f32)
    b_ih = np.asarray(b_ih, f32)
    b_hh = np.asarray(b_hh, f32)
    W_lin = np.asarray(W_lin, f32)
    b_lin = np.asarray(b_lin, f32)
    fwd_att = np.asarray(fwd_att, f32).reshape(-1)
    bwd_att = np.asarray(bwd_att, f32).reshape(-1)
    Lc = x.shape[1]

    gsl = slice(2 * H, 3 * H)
    wih_eff = W_ih.T.copy()
    wih_eff[:, gsl] *= 2.0
    whh_eff = (2.0 * W_hh).T.copy()
    whh_eff[:, gsl] *= 2.0
    bq = (b_ih + b_hh).copy()
    bq[gsl] *= 2.0
    wd = 2.0 * (W_lin[1] - W_lin[0])
    wdrep = np.empty((128, H), f32)
    for su in range(4):
        wdrep[:, 128 * su : 128 * (su + 1)] = wd[128 * su : 128 * (su + 1)][:, None]
    attrep = np.broadcast_to(
        np.concatenate([2.0 * fwd_att, 2.0 * bwd_att])[None, :], (128, 2 * H)
    ).copy()
    o = np.ones((128, 128), f32)
    tri = np.concatenate([np.triu(o, 1), np.tril(o, -1), o], axis=1)
    bdiff = float(b_lin[1] - b_lin[0])

    shared = {
        "wih": wih_eff.astype(bf16),
        "whh": whh_eff.astype(bf16),
        "biasq": bq[None, :].astype(bf16),
        "ones1": np.ones((1, 128), bf16),
        "wdrep": wdrep.astype(bf16),
        "attrep": attrep.astype(f32),
        "tri": tri.astype(bf16),
        "eye": np.eye(128).astype(bf16),
    }
    in_maps = []
    for c in range(NCORES):
        xs = x[2 * c : 2 * c + 2]
        xTc = np.ascontiguousarray(xs.transpose(2, 1, 0).reshape(D, BL * Lc))
        m = dict(shared)
        m["xT"] = xTc.astype(bf16)
        in_maps.append(m)
    return in_maps, bdiff


def kernel(x, W_ih, W_hh, b_ih, b_hh, W_lin, b_lin, fwd_att, bwd_att):
    in_maps, bdiff = _prep_inputs(
        x, W_ih, W_hh, b_ih, b_hh, W_lin, b_lin, fwd_att, bwd_att
    )
    key = ("full", bdiff)
    if key not in _NC_CACHE:
        _NC_CACHE[key] = build_nc(bdiff)
    nc = _NC_CACHE[key]
    res = bass_utils.run_bass_kernel_spmd(nc, in_maps, core_ids=list(range(NCORES)))
    out = np.concatenate([res.results[c]["out"] for c in range(NCORES)], axis=0)
    return np.ascontiguousarray(out.astype(np.float32))
